# revision 1
# baseline (speedup 1.0000x reference)
"""ASP (attentive statistics pooling) block kernel for Trainium2, 8 cores.

Shapes are hardcoded for the nn_ASPBlock problem:
  x: [32, 1536, 800] f32, W1: [128, 4608], W2: [1536, 128], A=128.
Sharding: data-parallel over batch (4 samples per core), params replicated.

Math (per sample b):
  mu_t = mean_T(x); sd_t = sqrt(clip(var_T(x, ddof=1), 1e-4))
  h    = tanh(bn1(relu(W1 @ [x; mu_t; sd_t] + b1)))
       = tanh(s1*relu(W1x@x + (W1m@mu_t + W1s@sd_t + b1)) + sh1)
  a    = bn2(relu(W2 @ h + b2)) ; w = softmax_T(a)
       softmax is shift-invariant per channel, so the bn2 shift drops out:
       w = softmax_T(s2 * relu(W2@h + b2))
  out  = [sum_T(x*w), sqrt(clip(sum_T(x^2*w) - mu^2, 1e-4))]
"""

import numpy as np

B, C, T, A = 32, 1536, 800, 128
N_CORES = 8
B_LOC = B // N_CORES          # 4 samples per core
NCH = C // 128                # 12 channel chunks
TS0 = 512                     # T split: 512 + 288 (one PSUM bank holds 512 f32)
BN_EPS = 1e-5
CLAMP = 1e-4
XG = 2                        # x chunks per DMA group
SQ_ON_ACT = 5                 # of the 12 sum(x^2) reductions, run this many on ACT
RSQRT_MAGIC = 0x5F3759DF

# set by test harness; when True the run is profiled and LAST_EXEC_NS is filled
TRACE = False
LAST_EXEC_NS = None

_BUILT = {}


def _rsqrt_newton(nc, mybir, pool, v, n_iters, tag):
    """y ~= 1/sqrt(v) elementwise on DVE (no ACT table needed).

    v: f32 SBUF AP [128, N], all elements >= CLAMP.
    """
    shp = list(v.shape)
    i32 = mybir.dt.int32
    f32 = mybir.dt.float32
    magic = pool.tile(shp, i32, name=f"{tag}_magic", bufs=2)
    nc.vector.memset(magic, RSQRT_MAGIC)
    sh = pool.tile(shp, i32, name=f"{tag}_sh", bufs=2)
    nc.vector.tensor_scalar(
        out=sh, in0=v.bitcast(i32), scalar1=1, scalar2=None,
        op0=mybir.AluOpType.arith_shift_right,
    )
    y = pool.tile(shp, f32, name=f"{tag}_y0", bufs=2)
    nc.vector.tensor_tensor(
        out=y.bitcast(i32), in0=magic, in1=sh, op=mybir.AluOpType.subtract
    )
    for it in range(n_iters):
        t = pool.tile(shp, f32, name=f"{tag}_t{it}", bufs=2)
        nc.vector.tensor_tensor(out=t, in0=v, in1=y, op=mybir.AluOpType.mult)
        nc.vector.tensor_tensor(out=t, in0=t, in1=y, op=mybir.AluOpType.mult)
        # f = 1.5 - 0.5*t
        nc.vector.tensor_scalar(
            out=t, in0=t, scalar1=-0.5, scalar2=1.5,
            op0=mybir.AluOpType.mult, op1=mybir.AluOpType.add,
        )
        y2 = pool.tile(shp, f32, name=f"{tag}_y{it + 1}", bufs=2)
        nc.vector.tensor_tensor(out=y2, in0=y, in1=t, op=mybir.AluOpType.mult)
        y = y2
    return y


def build_kernel():
    import concourse.bacc as bacc
    import concourse.tile as tile
    from concourse import mybir

    f32 = mybir.dt.float32
    bf16 = mybir.dt.bfloat16
    ALU = mybir.AluOpType
    ACTF = mybir.ActivationFunctionType

    nc = bacc.Bacc()

    x_d = nc.dram_tensor("x_in", [B_LOC, C, T], f32, kind="ExternalInput")
    w1x_d = nc.dram_tensor("w1xT", [128, NCH, A], bf16, kind="ExternalInput")
    w1m_d = nc.dram_tensor("w1mT", [128, NCH, A], f32, kind="ExternalInput")
    w1s_d = nc.dram_tensor("w1sT", [128, NCH, A], f32, kind="ExternalInput")
    w2_d = nc.dram_tensor("w2T", [A, C], bf16, kind="ExternalInput")
    b1_d = nc.dram_tensor("b1v", [A, 1], f32, kind="ExternalInput")
    s1_d = nc.dram_tensor("s1v", [A, 1], f32, kind="ExternalInput")
    sh1_d = nc.dram_tensor("sh1v", [A, 1], f32, kind="ExternalInput")
    b2_d = nc.dram_tensor("b2c", [128, NCH], f32, kind="ExternalInput")
    s2_d = nc.dram_tensor("s2c", [128, NCH], f32, kind="ExternalInput")
    out_d = nc.dram_tensor("out_asp", [B_LOC, 2 * C], f32, kind="ExternalOutput")

    with tile.TileContext(nc) as tc:
        with (
            tc.tile_pool(name="consts", bufs=1) as consts,
            tc.tile_pool(name="xf", bufs=3) as xfp,
            tc.tile_pool(name="xbf", bufs=2) as xbp,
            tc.tile_pool(name="hp", bufs=2) as hp,
            tc.tile_pool(name="cp", bufs=3) as cp,
            tc.tile_pool(name="st", bufs=2) as st,
            tc.tile_pool(name="ph", bufs=1, space="PSUM") as php,
            tc.tile_pool(name="pa", bufs=2, space="PSUM") as pap,
            tc.tile_pool(name="pv", bufs=2, space="PSUM") as pvp,
        ):
            # ---- replicated params into SBUF
            w1x_sb = consts.tile([128, NCH, A], bf16)
            nc.sync.dma_start(out=w1x_sb, in_=w1x_d[:, :, :])
            w1m_sb = consts.tile([128, NCH, A], f32)
            nc.sync.dma_start(out=w1m_sb, in_=w1m_d[:, :, :])
            w1s_sb = consts.tile([128, NCH, A], f32)
            nc.sync.dma_start(out=w1s_sb, in_=w1s_d[:, :, :])
            w2_sb = consts.tile([A, C], bf16)
            nc.sync.dma_start(out=w2_sb, in_=w2_d[:, :])
            b1_sb = consts.tile([A, 1], f32)
            nc.sync.dma_start(out=b1_sb, in_=b1_d[:, :])
            s1_sb = consts.tile([A, 1], f32)
            nc.sync.dma_start(out=s1_sb, in_=s1_d[:, :])
            sh1_sb = consts.tile([A, 1], f32)
            nc.sync.dma_start(out=sh1_sb, in_=sh1_d[:, :])
            b2_sb = consts.tile([128, NCH], f32)
            nc.sync.dma_start(out=b2_sb, in_=b2_d[:, :])
            s2_sb = consts.tile([128, NCH], f32)
            nc.sync.dma_start(out=s2_sb, in_=s2_d[:, :])

            # batched accumulators for the whole core (col = b*NCH + k)
            sva = consts.tile([128, B_LOC * NCH], f32)
            m1a = consts.tile([128, B_LOC * NCH], f32)
            m2a = consts.tile([128, B_LOC * NCH], f32)

            for b in range(B_LOC):
                # ---------- phase A: load x, convert to bf16, stats ----------
                xv = x_d[b].rearrange("(k p) t -> p k t", p=128)
                xbf = xbp.tile([128, NCH, T], bf16, name="xbf", tag="xbf")
                sx = st.tile([128, NCH], f32, name="sx", tag="sx")
                sxx = st.tile([128, NCH], f32, name="sxx", tag="sxx")
                for g in range(NCH // XG):
                    xf = xfp.tile([128, XG, T], f32, name="xf", tag="xf")
                    nc.sync.dma_start(
                        out=xf, in_=xv[:, g * XG:(g + 1) * XG, :]
                    )
                    for j in range(XG):
                        k = g * XG + j
                        # bf16 convert + sum(x) in one pass (DVE)
                        nc.vector.tensor_scalar(
                            out=xbf[:, k, :], in0=xf[:, j, :],
                            scalar1=1.0, scalar2=None,
                            op0=ALU.mult, op1=ALU.add,
                            accum_out=sx[:, k:k + 1],
                        )
                        # sum(x^2): split between ACT (from f32) and DVE (bf16)
                        if k < SQ_ON_ACT:
                            xsq = cp.tile([128, T], bf16, name="xsq", tag="xsq")
                            nc.scalar.activation(
                                out=xsq, in_=xf[:, j, :], func=ACTF.Square,
                                accum_out=sxx[:, k:k + 1],
                            )
                        else:
                            xsq = cp.tile([128, T], bf16, name="xsq", tag="xsq")
                            nc.vector.scalar_tensor_tensor(
                                out=xsq, in0=xbf[:, k, :], scalar=1.0,
                                in1=xbf[:, k, :],
                                op0=ALU.mult, op1=ALU.mult,
                                accum_out=sxx[:, k:k + 1],
                            )

                # ---------- phase A': sd_t = sqrt(clip(var, CLAMP)) ----------
                var = st.tile([128, NCH], f32, name="var", tag="var")
                nc.vector.tensor_tensor(out=var, in0=sx, in1=sx, op=ALU.mult)
                # var = sxx - (sx*sx)/T  (as (sx2 * -1/T) + sxx)
                nc.vector.scalar_tensor_tensor(
                    out=var, in0=var, scalar=-1.0 / T, in1=sxx,
                    op0=ALU.mult, op1=ALU.add,
                )
                nc.vector.tensor_scalar(
                    out=var, in0=var, scalar1=1.0 / (T - 1), scalar2=CLAMP,
                    op0=ALU.mult, op1=ALU.max,
                )
                rq = _rsqrt_newton(nc, mybir, st, var, 1, tag="sdn")
                sd = st.tile([128, NCH], f32, name="sd", tag="sd")
                nc.vector.tensor_tensor(out=sd, in0=var, in1=rq, op=ALU.mult)

                # ---------- matvec: hv = W1m@(sx/T) + W1s@sd  (PE) ----------
                phv = pvp.tile([128, 2], f32, name="phv", tag="phv")
                for k in range(NCH):
                    nc.tensor.matmul(
                        phv[:, 0:1], w1m_sb[:, k, :], sx[:, k:k + 1],
                        start=(k == 0), stop=False,
                    )
                for k in range(NCH):
                    nc.tensor.matmul(
                        phv[:, 0:1], w1s_sb[:, k, :], sd[:, k:k + 1],
                        start=False, stop=(k == NCH - 1),
                    )
                hb = st.tile([A, 1], f32, name="hb", tag="hb")
                nc.vector.tensor_scalar(
                    out=hb, in0=phv[:, 0:1], scalar1=b1_sb[:, 0:1], scalar2=None,
                    op0=ALU.add,
                )

                # ---------- phase B: h = tanh(s1*relu(mm1 + hv) + sh1) ----------
                ph = php.tile([128, 1024], f32, name="ph", tag="ph")
                for k in range(NCH):
                    nc.tensor.matmul(
                        ph[:, 0:TS0], w1x_sb[:, k, :], xbf[:, k, 0:TS0],
                        start=(k == 0), stop=(k == NCH - 1),
                    )
                    nc.tensor.matmul(
                        ph[:, TS0:T], w1x_sb[:, k, :], xbf[:, k, TS0:T],
                        start=(k == 0), stop=(k == NCH - 1),
                    )
                r1 = hp.tile([128, T], bf16, name="r1", tag="r1")
                nc.scalar.activation(
                    out=r1, in_=ph[:, 0:T], func=ACTF.Relu, bias=hb[:, 0:1],
                )
                h = hp.tile([128, T], bf16, name="h", tag="h")
                nc.scalar.activation(
                    out=h, in_=r1, func=ACTF.Tanh,
                    bias=sh1_sb[:, 0:1], scale=s1_sb[:, 0:1],
                )

                # ---------- phase C: per channel chunk ----------
                for k in range(NCH):
                    col = b * NCH + k
                    pa = pap.tile([128, 1024], f32, name="pa", tag="pa")
                    lhsT = w2_sb[:, k * 128:(k + 1) * 128]
                    nc.tensor.matmul(
                        pa[:, 0:TS0], lhsT, h[:, 0:TS0], start=True, stop=True
                    )
                    nc.tensor.matmul(
                        pa[:, TS0:T], lhsT, h[:, TS0:T], start=True, stop=True
                    )
                    rt = cp.tile([128, T], bf16, name="rt", tag="rt")
                    nc.scalar.activation(
                        out=rt, in_=pa[:, 0:T], func=ACTF.Relu,
                        bias=b2_sb[:, k:k + 1],
                    )
                    p = cp.tile([128, T], bf16, name="p", tag="p")
                    nc.scalar.activation(
                        out=p, in_=rt, func=ACTF.Exp, scale=s2_sb[:, k:k + 1],
                        accum_out=sva[:, col:col + 1],
                    )
                    t1 = cp.tile([128, T], bf16, name="t1", tag="t1")
                    nc.vector.scalar_tensor_tensor(
                        out=t1, in0=p, scalar=1.0, in1=xbf[:, k, :],
                        op0=ALU.mult, op1=ALU.mult,
                        accum_out=m1a[:, col:col + 1],
                    )
                    t2 = cp.tile([128, T], bf16, name="t2", tag="t2")
                    nc.vector.scalar_tensor_tensor(
                        out=t2, in0=t1, scalar=1.0, in1=xbf[:, k, :],
                        op0=ALU.mult, op1=ALU.mult,
                        accum_out=m2a[:, col:col + 1],
                    )

            # ---------- finals (batched over all samples) ----------
            nch4 = B_LOC * NCH
            rs = consts.tile([128, nch4], f32)
            nc.vector.reciprocal(out=rs, in_=sva)
            mua = consts.tile([128, nch4], f32)
            nc.vector.tensor_tensor(out=mua, in0=m1a, in1=rs, op=ALU.mult)
            e2a = consts.tile([128, nch4], f32)
            nc.vector.tensor_tensor(out=e2a, in0=m2a, in1=rs, op=ALU.mult)
            msq = consts.tile([128, nch4], f32)
            nc.vector.tensor_tensor(out=msq, in0=mua, in1=mua, op=ALU.mult)
            dv = consts.tile([128, nch4], f32)
            nc.vector.tensor_tensor(out=dv, in0=e2a, in1=msq, op=ALU.subtract)
            nc.vector.tensor_scalar(
                out=dv, in0=dv, scalar1=CLAMP, scalar2=None, op0=ALU.max
            )
            rqf = _rsqrt_newton(nc, mybir, st, dv, 2, tag="sgn")
            sga = consts.tile([128, nch4], f32)
            nc.vector.tensor_tensor(out=sga, in0=dv, in1=rqf, op=ALU.mult)

            for b in range(B_LOC):
                nc.sync.dma_start(
                    out=out_d[b, 0:C].rearrange("(k p) -> p k", p=128),
                    in_=mua[:, b * NCH:(b + 1) * NCH],
                )
                nc.sync.dma_start(
                    out=out_d[b, C:2 * C].rearrange("(k p) -> p k", p=128),
                    in_=sga[:, b * NCH:(b + 1) * NCH],
                )

    nc.compile()
    return nc


def _prep_params(W1, b1, gamma1, beta1, mean1, var1, W2, b2, gamma2, beta2,
                 mean2, var2):
    import ml_dtypes

    bf16 = ml_dtypes.bfloat16
    f32 = np.float32
    W1 = np.asarray(W1, f32)
    W2 = np.asarray(W2, f32)
    s1 = np.asarray(gamma1, f32) / np.sqrt(np.asarray(var1, f32) + BN_EPS)
    sh1 = np.asarray(beta1, f32) - np.asarray(mean1, f32) * s1
    s2 = np.asarray(gamma2, f32) / np.sqrt(np.asarray(var2, f32) + BN_EPS)

    def chunkT(w, scale=1.0):
        # w: [A, C] -> [128, NCH, A] where [p, k, :] = w[:, k*128+p] * scale
        wt = np.ascontiguousarray(
            (w.T * scale).reshape(NCH, 128, A).transpose(1, 0, 2)
        )
        return wt

    w1xT = chunkT(W1[:, :C]).astype(bf16)
    w1mT = chunkT(W1[:, C:2 * C], 1.0 / T).astype(f32)
    w1sT = chunkT(W1[:, 2 * C:]).astype(f32)
    w2T = np.ascontiguousarray(W2.T).astype(bf16)
    b2c = np.ascontiguousarray(np.asarray(b2, f32).reshape(NCH, 128).T)
    s2c = np.ascontiguousarray(s2.reshape(NCH, 128).T)
    return {
        "w1xT": w1xT,
        "w1mT": w1mT,
        "w1sT": w1sT,
        "w2T": w2T,
        "b1v": np.asarray(b1, f32).reshape(A, 1),
        "s1v": s1.reshape(A, 1),
        "sh1v": sh1.reshape(A, 1),
        "b2c": b2c,
        "s2c": s2c,
    }


def kernel(x, W1, b1, gamma1, beta1, mean1, var1,
           W2, b2, gamma2, beta2, mean2, var2):
    global LAST_EXEC_NS
    from concourse.bass_utils import run_bass_kernel_spmd

    if "nc" not in _BUILT:
        _BUILT["nc"] = build_kernel()
    nc = _BUILT["nc"]

    x = np.ascontiguousarray(np.asarray(x, np.float32))
    params = _prep_params(W1, b1, gamma1, beta1, mean1, var1,
                          W2, b2, gamma2, beta2, mean2, var2)
    in_maps = []
    for i in range(N_CORES):
        m = dict(params)
        m["x_in"] = np.ascontiguousarray(x[i * B_LOC:(i + 1) * B_LOC])
        in_maps.append(m)

    res = run_bass_kernel_spmd(nc, in_maps, list(range(N_CORES)), trace=TRACE)
    LAST_EXEC_NS = res.exec_time_ns
    out = np.concatenate(
        [res.results[i]["out_asp"] for i in range(N_CORES)], axis=0
    )
    return out.astype(np.float32)


# revision 4
# speedup vs baseline: 1.0923x; 1.0923x over previous
"""ASP (attentive statistics pooling) block kernel for Trainium2, 8 cores.

Shapes hardcoded for nn_ASPBlock: x [32, 1536, 800] f32, W1 [128, 4608],
W2 [1536, 128], A=128. Data-parallel over batch: 4 samples per core.

Channel layout is "dense": channel c lives at (partition p, chunk j) with
c = p*12 + j, so each partition's 12 channels are contiguous in DRAM and
one DMA descriptor moves 38.4KB.

Math (per sample):
  mu_t = mean_T(x); sd_t = sqrt(clip(var_T(x, ddof=1), 1e-4))
  hv   = W1m@mu_t + W1s@sd_t + b1            (stats matvec, M=1 matmuls)
  h    = tanh(s1*relu(W1x@x + hv) + sh1)     (hv injected via K=1 bias matmul)
  softmax per channel over T is shift invariant, so with s2>0:
  w    = softmax_T(s2*relu(W2@h + b2)) ; p_raw = max(exp(s2*(W2@h) + s2*b2), 1)
  out  = [m1/s, sqrt(clip(m2/s - (m1/s)^2, 1e-4))],
         s = sum p_raw, m1 = sum p_raw*x, m2 = sum p_raw*x^2
"""

import numpy as np

B, C, T, A = 32, 1536, 800, 128
N_CORES = 8
B_LOC = B // N_CORES          # 4 samples per core
NCH = C // 128                # 12 chunks; channel c = p*NCH + j
TS0 = 512
BN_EPS = 1e-5
CLAMP = 1e-4
RSQRT_MAGIC = 0x5F3759DF

SA = 4      # chunks per sample whose x-stats run on ACT (rest: DVE bn_stats)
SB = 10     # chunks per sample whose softmax-denominator runs on ACT

TRACE = False
LAST_EXEC_NS = None
_BUILT = {}


def _rsqrt_newton(nc, mybir, pool, v, n_iters, tag):
    """y ~= 1/sqrt(v) on DVE (no ACT sqrt table). v f32 >= CLAMP."""
    shp = list(v.shape)
    i32 = mybir.dt.int32
    f32 = mybir.dt.float32
    ALU = mybir.AluOpType
    magic = pool.tile(shp, i32, name=f"{tag}_magic", bufs=2)
    nc.vector.memset(magic, RSQRT_MAGIC)
    sh = pool.tile(shp, i32, name=f"{tag}_sh", bufs=2)
    nc.vector.tensor_scalar(out=sh, in0=v.bitcast(i32), scalar1=1, scalar2=None,
                            op0=ALU.arith_shift_right)
    y = pool.tile(shp, f32, name=f"{tag}_y0", bufs=2)
    nc.vector.tensor_tensor(out=y.bitcast(i32), in0=magic, in1=sh,
                            op=ALU.subtract)
    for it in range(n_iters):
        t = pool.tile(shp, f32, name=f"{tag}_t{it}", bufs=2)
        nc.vector.tensor_tensor(out=t, in0=v, in1=y, op=ALU.mult)
        nc.vector.tensor_tensor(out=t, in0=t, in1=y, op=ALU.mult)
        nc.vector.tensor_scalar(out=t, in0=t, scalar1=-0.5, scalar2=1.5,
                                op0=ALU.mult, op1=ALU.add)
        y2 = pool.tile(shp, f32, name=f"{tag}_y{it + 1}", bufs=2)
        nc.vector.tensor_tensor(out=y2, in0=y, in1=t, op=ALU.mult)
        y = y2
    return y


def build_kernel():
    import concourse.bacc as bacc
    import concourse.tile as tile
    from concourse import mybir

    f32 = mybir.dt.float32
    bf16 = mybir.dt.bfloat16
    ALU = mybir.AluOpType
    ACTF = mybir.ActivationFunctionType

    nc = bacc.Bacc()

    x_d = nc.dram_tensor("x_in", [B_LOC, C, T], f32, kind="ExternalInput")
    w1x_d = nc.dram_tensor("w1xg", [128, NCH, A], f32, kind="ExternalInput")
    w1m_d = nc.dram_tensor("w1mg", [128, NCH, A], f32, kind="ExternalInput")
    w1s_d = nc.dram_tensor("w1sg", [128, NCH, A], f32, kind="ExternalInput")
    w2_d = nc.dram_tensor("w2g", [A, NCH, 128], bf16, kind="ExternalInput")
    b1t_d = nc.dram_tensor("b1T", [1, A], f32, kind="ExternalInput")
    s1_d = nc.dram_tensor("s1v", [A, 1], f32, kind="ExternalInput")
    sh1_d = nc.dram_tensor("sh1v", [A, 1], f32, kind="ExternalInput")
    s2_d = nc.dram_tensor("s2c", [128, NCH], f32, kind="ExternalInput")
    s2b2_d = nc.dram_tensor("s2b2c", [128, NCH], f32, kind="ExternalInput")
    out_d = nc.dram_tensor("out_asp", [B_LOC, 2 * C], f32, kind="ExternalOutput")

    with tile.TileContext(nc) as tc:
        with (
            tc.tile_pool(name="consts", bufs=1) as consts,
            tc.tile_pool(name="xf", bufs=2) as xfp,
            tc.tile_pool(name="hp", bufs=2) as hp,
            tc.tile_pool(name="cp", bufs=3) as cp,
            tc.tile_pool(name="st", bufs=2) as st,
            tc.tile_pool(name="ph", bufs=1, space="PSUM") as php,
            tc.tile_pool(name="pa", bufs=2, space="PSUM") as pap,
            tc.tile_pool(name="pv", bufs=2, space="PSUM") as pvp,
        ):
            w1x_sb = consts.tile([128, NCH, A], f32)
            nc.sync.dma_start(out=w1x_sb, in_=w1x_d[:, :, :])
            w1m_sb = consts.tile([128, NCH, A], f32)
            nc.sync.dma_start(out=w1m_sb, in_=w1m_d[:, :, :])
            w1s_sb = consts.tile([128, NCH, A], f32)
            nc.sync.dma_start(out=w1s_sb, in_=w1s_d[:, :, :])
            w2_sb = consts.tile([A, NCH, 128], bf16)
            nc.sync.dma_start(out=w2_sb, in_=w2_d[:, :, :])
            b1t_sb = consts.tile([1, A], f32)
            nc.sync.dma_start(out=b1t_sb, in_=b1t_d[:, :])
            s1_sb = consts.tile([A, 1], f32)
            nc.sync.dma_start(out=s1_sb, in_=s1_d[:, :])
            sh1_sb = consts.tile([A, 1], f32)
            nc.sync.dma_start(out=sh1_sb, in_=sh1_d[:, :])
            s2_sb = consts.tile([128, NCH], f32)
            nc.sync.dma_start(out=s2_sb, in_=s2_d[:, :])
            s2b2_sb = consts.tile([128, NCH], f32)
            nc.sync.dma_start(out=s2b2_sb, in_=s2b2_d[:, :])
            onesT = consts.tile([1, T], f32)
            nc.vector.memset(onesT, 1.0)
            neg1 = consts.tile([128, 1], f32)
            nc.vector.memset(neg1, -1.0)

            nch4 = B_LOC * NCH
            sva = consts.tile([128, nch4], f32)
            m1a = consts.tile([128, nch4], f32)
            m2a = consts.tile([128, nch4], f32)

            for b in range(B_LOC):
                # ---- phase A: one DMA for the whole sample ----
                xf = xfp.tile([128, NCH, T], f32, name="xf", tag="xf")
                nc.sync.dma_start(
                    out=xf, in_=x_d[b].rearrange("(p j) t -> p j t", j=NCH)
                )

                # ---- x stats: mean and population var per chunk ----
                mv = st.tile([128, NCH, 2], f32, name="mv", tag="mv")
                if SA > 0:
                    sxa = st.tile([128, SA], f32, name="sxa", tag="sxa")
                    sxxa = st.tile([128, SA], f32, name="sxxa", tag="sxxa")
                for j in range(NCH):
                    if j < SA:
                        tr1 = cp.tile([128, T], bf16, name="tr1", tag="xsq")
                        nc.scalar.activation(out=tr1, in_=xf[:, j, :],
                                             func=ACTF.Identity,
                                             accum_out=sxa[:, j:j + 1])
                        tr2 = cp.tile([128, T], bf16, name="tr2", tag="xsq")
                        nc.scalar.activation(out=tr2, in_=xf[:, j, :],
                                             func=ACTF.Square,
                                             accum_out=sxxa[:, j:j + 1])
                    else:
                        st6 = st.tile([128, 2, 6], f32, name="st6", tag="st6",
                                      bufs=3)
                        nc.vector.bn_stats(out=st6[:, 0, :], in_=xf[:, j, 0:TS0])
                        nc.vector.bn_stats(out=st6[:, 1, :], in_=xf[:, j, TS0:T])
                        nc.vector.bn_aggr(out=mv[:, j, :], in_=st6)
                if SA > 0:
                    # mean = sx/T ; var_pop = sxx/T - mean^2
                    nc.vector.tensor_scalar(out=mv[:, 0:SA, 0], in0=sxa,
                                            scalar1=1.0 / T, scalar2=None,
                                            op0=ALU.mult)
                    msq = st.tile([128, SA], f32, name="msq", tag="msq")
                    nc.vector.tensor_tensor(out=msq, in0=mv[:, 0:SA, 0],
                                            in1=mv[:, 0:SA, 0], op=ALU.mult)
                    nc.vector.scalar_tensor_tensor(
                        out=mv[:, 0:SA, 1], in0=sxxa, scalar=1.0 / T, in1=msq,
                        op0=ALU.mult, op1=ALU.subtract)

                # ---- sd = sqrt(clip(var_pop*T/(T-1), CLAMP)) ----
                sdsq = st.tile([128, NCH], f32, name="sdsq", tag="sdsq")
                nc.vector.tensor_scalar(out=sdsq, in0=mv[:, :, 1],
                                        scalar1=float(T) / (T - 1),
                                        scalar2=CLAMP,
                                        op0=ALU.mult, op1=ALU.max)
                rqs = _rsqrt_newton(nc, mybir, st, sdsq, 1, tag="sdn")
                sd = st.tile([128, NCH], f32, name="sd", tag="sd")
                nc.vector.tensor_tensor(out=sd, in0=sdsq, in1=rqs, op=ALU.mult)

                # ---- stats matvec: hvT[1, A] = mu@W1m + sd@W1s (M=1) ----
                hvt = pvp.tile([1, A], f32, name="hvt", tag="hvt")
                for j in range(NCH):
                    nc.tensor.matmul(hvt[0:1, :], mv[:, j, 0:1], w1m_sb[:, j, :],
                                     start=(j == 0), stop=False)
                for j in range(NCH):
                    nc.tensor.matmul(hvt[0:1, :], sd[:, j:j + 1], w1s_sb[:, j, :],
                                     start=False, stop=(j == NCH - 1))
                hvb = st.tile([1, A], f32, name="hvb", tag="hvb")
                nc.vector.tensor_tensor(out=hvb, in0=hvt[0:1, :],
                                        in1=b1t_sb[0:1, :], op=ALU.add)

                # ---- mm1 (+ K=1 bias matmul) and h ----
                ph = php.tile([128, 1024], f32, name="ph", tag="ph")
                nc.tensor.matmul(ph[:, 0:TS0], hvb[0:1, :], onesT[0:1, 0:TS0],
                                 start=True, stop=False)
                nc.tensor.matmul(ph[:, TS0:T], hvb[0:1, :], onesT[0:1, 0:T - TS0],
                                 start=True, stop=False)
                for j in range(NCH):
                    nc.tensor.matmul(ph[:, 0:TS0], w1x_sb[:, j, :],
                                     xf[:, j, 0:TS0],
                                     start=False, stop=(j == NCH - 1))
                    nc.tensor.matmul(ph[:, TS0:T], w1x_sb[:, j, :],
                                     xf[:, j, TS0:T],
                                     start=False, stop=(j == NCH - 1))
                r1 = hp.tile([128, T], bf16, name="r1", tag="r1")
                nc.scalar.activation(out=r1, in_=ph[:, 0:T], func=ACTF.Relu)
                h = hp.tile([128, T], bf16, name="h", tag="h")
                nc.scalar.activation(out=h, in_=r1, func=ACTF.Tanh,
                                     bias=sh1_sb[:, 0:1], scale=s1_sb[:, 0:1])

                # ---- phase C ----
                for j in range(NCH):
                    col = b * NCH + j
                    pa = pap.tile([128, 1024], f32, name="pa", tag="pa")
                    nc.tensor.matmul(pa[:, 0:TS0], w2_sb[:, j, :], h[:, 0:TS0],
                                     start=True, stop=True)
                    nc.tensor.matmul(pa[:, TS0:T], w2_sb[:, j, :], h[:, TS0:T],
                                     start=True, stop=True)
                    # e = exp(s2*a + s2*b2); p_raw = max(e, 1) folded below
                    e = cp.tile([128, T], bf16, name="e", tag="e")
                    nc.scalar.activation(out=e, in_=pa[:, 0:T], func=ACTF.Exp,
                                         bias=s2b2_sb[:, j:j + 1],
                                         scale=s2_sb[:, j:j + 1])
                    if j < NCH - SB:
                        # s on DVE: sva col = sum(max(e,1))
                        ptr = cp.tile([128, T], bf16, name="ptr", tag="xsq")
                        nc.vector.tensor_scalar(
                            out=ptr, in0=e, scalar1=1.0, scalar2=None,
                            op0=ALU.max, op1=ALU.add,
                            accum_out=sva[:, col:col + 1])
                    else:
                        # s on ACT: sva col = sum(relu(e-1)) = s - T
                        ptr = cp.tile([128, T], bf16, name="ptr", tag="xsq")
                        nc.scalar.activation(out=ptr, in_=e, func=ACTF.Relu,
                                             bias=neg1[:, 0:1],
                                             accum_out=sva[:, col:col + 1])
                    t1 = cp.tile([128, T], bf16, name="t1", tag="t1")
                    nc.vector.scalar_tensor_tensor(
                        out=t1, in0=e, scalar=1.0, in1=xf[:, j, :],
                        op0=ALU.max, op1=ALU.mult,
                        accum_out=m1a[:, col:col + 1])
                    t2 = cp.tile([128, T], bf16, name="t2", tag="t2")
                    nc.vector.scalar_tensor_tensor(
                        out=t2, in0=t1, scalar=1.0, in1=xf[:, j, :],
                        op0=ALU.mult, op1=ALU.mult,
                        accum_out=m2a[:, col:col + 1])

            # ---- finals (batched) ----
            if SB > 0:
                for b in range(B_LOC):
                    c0 = b * NCH + (NCH - SB)
                    c1 = (b + 1) * NCH
                    nc.vector.tensor_scalar(out=sva[:, c0:c1],
                                            in0=sva[:, c0:c1],
                                            scalar1=float(T), scalar2=None,
                                            op0=ALU.add)
            rs = consts.tile([128, nch4], f32)
            nc.vector.reciprocal(out=rs, in_=sva)
            mua = consts.tile([128, nch4], f32)
            nc.vector.tensor_tensor(out=mua, in0=m1a, in1=rs, op=ALU.mult)
            e2a = consts.tile([128, nch4], f32)
            nc.vector.tensor_tensor(out=e2a, in0=m2a, in1=rs, op=ALU.mult)
            msqa = consts.tile([128, nch4], f32)
            nc.vector.tensor_tensor(out=msqa, in0=mua, in1=mua, op=ALU.mult)
            dv = consts.tile([128, nch4], f32)
            nc.vector.tensor_tensor(out=dv, in0=e2a, in1=msqa, op=ALU.subtract)
            nc.vector.tensor_scalar(out=dv, in0=dv, scalar1=CLAMP, scalar2=None,
                                    op0=ALU.max)
            rqf = _rsqrt_newton(nc, mybir, st, dv, 2, tag="sgn")
            sga = consts.tile([128, nch4], f32)
            nc.vector.tensor_tensor(out=sga, in0=dv, in1=rqf, op=ALU.mult)

            for b in range(B_LOC):
                nc.sync.dma_start(
                    out=out_d[b, 0:C].rearrange("(p j) -> p j", j=NCH),
                    in_=mua[:, b * NCH:(b + 1) * NCH],
                )
                nc.sync.dma_start(
                    out=out_d[b, C:2 * C].rearrange("(p j) -> p j", j=NCH),
                    in_=sga[:, b * NCH:(b + 1) * NCH],
                )

    nc.compile()
    return nc


def _prep_params(W1, b1, gamma1, beta1, mean1, var1, W2, b2, gamma2, beta2,
                 mean2, var2):
    import ml_dtypes

    bf16 = ml_dtypes.bfloat16
    f32 = np.float32
    W1 = np.asarray(W1, f32)
    W2 = np.asarray(W2, f32)
    s1 = np.asarray(gamma1, f32) / np.sqrt(np.asarray(var1, f32) + BN_EPS)
    sh1 = np.asarray(beta1, f32) - np.asarray(mean1, f32) * s1
    s2 = np.asarray(gamma2, f32) / np.sqrt(np.asarray(var2, f32) + BN_EPS)
    assert (s2 > 0).all(), "kernel fast path requires positive bn2 scale"
    b2 = np.asarray(b2, f32)

    w1xg = np.ascontiguousarray(W1[:, :C].T.reshape(128, NCH, A))
    w1mg = np.ascontiguousarray(W1[:, C:2 * C].T.reshape(128, NCH, A))
    w1sg = np.ascontiguousarray(W1[:, 2 * C:].T.reshape(128, NCH, A))
    w2g = np.ascontiguousarray(
        W2.reshape(128, NCH, A).transpose(2, 1, 0)).astype(bf16)
    return {
        "w1xg": w1xg,
        "w1mg": w1mg,
        "w1sg": w1sg,
        "w2g": w2g,
        "b1T": np.asarray(b1, f32).reshape(1, A),
        "s1v": s1.reshape(A, 1),
        "sh1v": sh1.reshape(A, 1),
        "s2c": np.ascontiguousarray(s2.reshape(128, NCH)),
        "s2b2c": np.ascontiguousarray((s2 * b2).reshape(128, NCH)),
    }


def kernel(x, W1, b1, gamma1, beta1, mean1, var1,
           W2, b2, gamma2, beta2, mean2, var2):
    global LAST_EXEC_NS
    from concourse.bass_utils import run_bass_kernel_spmd

    if "nc" not in _BUILT:
        _BUILT["nc"] = build_kernel()
    nc = _BUILT["nc"]

    x = np.ascontiguousarray(np.asarray(x, np.float32))
    params = _prep_params(W1, b1, gamma1, beta1, mean1, var1,
                          W2, b2, gamma2, beta2, mean2, var2)
    in_maps = []
    for i in range(N_CORES):
        m = dict(params)
        m["x_in"] = np.ascontiguousarray(x[i * B_LOC:(i + 1) * B_LOC])
        in_maps.append(m)

    res = run_bass_kernel_spmd(nc, in_maps, list(range(N_CORES)), trace=TRACE)
    LAST_EXEC_NS = res.exec_time_ns
    out = np.concatenate(
        [res.results[i]["out_asp"] for i in range(N_CORES)], axis=0
    )
    return out.astype(np.float32)


# revision 5
# speedup vs baseline: 1.1362x; 1.0402x over previous
"""ASP (attentive statistics pooling) block kernel for Trainium2, 8 cores.

Shapes hardcoded for nn_ASPBlock: x [32, 1536, 800] f32, W1 [128, 4608],
W2 [1536, 128], A=128. Data-parallel over batch: 4 samples per core.

Channel layout is "dense": channel c lives at (partition p, chunk j) with
c = p*12 + j, so each partition's 12 channels are contiguous in DRAM and
one DMA descriptor moves 38.4KB.

Math (per sample):
  mu_t = mean_T(x); sd_t = sqrt(clip(var_T(x, ddof=1), 1e-4))
  hv   = W1m@mu_t + W1s@sd_t + b1            (stats matvec, M=1 matmuls)
  h    = tanh(s1*relu(W1x@x + hv) + sh1)     (hv injected via K=1 bias matmul)
  softmax per channel over T is shift invariant, so with s2>0:
  w    = softmax_T(s2*relu(W2@h + b2)) ; p_raw = max(exp(s2*(W2@h) + s2*b2), 1)
  out  = [m1/s, sqrt(clip(m2/s - (m1/s)^2, 1e-4))],
         s = sum p_raw, m1 = sum p_raw*x, m2 = sum p_raw*x^2
"""

import numpy as np

B, C, T, A = 32, 1536, 800, 128
N_CORES = 8
B_LOC = B // N_CORES          # 4 samples per core
NCH = C // 128                # 12 chunks; channel c = p*NCH + j
TS0 = 512
BN_EPS = 1e-5
CLAMP = 1e-4
RSQRT_MAGIC = 0x5F3759DF

SA = 4      # chunks per sample whose x-stats run on ACT (rest: DVE bn_stats)
SB = 10     # chunks per sample whose softmax-denominator runs on ACT

TRACE = False
LAST_EXEC_NS = None
_BUILT = {}


def _rsqrt_newton(nc, mybir, pool, v, n_iters, tag):
    """y ~= 1/sqrt(v) on DVE (no ACT sqrt table). v f32 >= CLAMP."""
    shp = list(v.shape)
    i32 = mybir.dt.int32
    f32 = mybir.dt.float32
    ALU = mybir.AluOpType
    magic = pool.tile(shp, i32, name=f"{tag}_magic", bufs=2)
    nc.vector.memset(magic, RSQRT_MAGIC)
    sh = pool.tile(shp, i32, name=f"{tag}_sh", bufs=2)
    nc.vector.tensor_scalar(out=sh, in0=v.bitcast(i32), scalar1=1, scalar2=None,
                            op0=ALU.arith_shift_right)
    y = pool.tile(shp, f32, name=f"{tag}_y0", bufs=2)
    nc.vector.tensor_tensor(out=y.bitcast(i32), in0=magic, in1=sh,
                            op=ALU.subtract)
    for it in range(n_iters):
        t = pool.tile(shp, f32, name=f"{tag}_t{it}", bufs=2)
        nc.vector.tensor_tensor(out=t, in0=v, in1=y, op=ALU.mult)
        nc.vector.tensor_tensor(out=t, in0=t, in1=y, op=ALU.mult)
        nc.vector.tensor_scalar(out=t, in0=t, scalar1=-0.5, scalar2=1.5,
                                op0=ALU.mult, op1=ALU.add)
        y2 = pool.tile(shp, f32, name=f"{tag}_y{it + 1}", bufs=2)
        nc.vector.tensor_tensor(out=y2, in0=y, in1=t, op=ALU.mult)
        y = y2
    return y


def build_kernel():
    import concourse.bacc as bacc
    import concourse.tile as tile
    from concourse import mybir

    f32 = mybir.dt.float32
    bf16 = mybir.dt.bfloat16
    ALU = mybir.AluOpType
    ACTF = mybir.ActivationFunctionType

    nc = bacc.Bacc()

    x_d = nc.dram_tensor("x_in", [B_LOC, C, T], f32, kind="ExternalInput")
    w1x_d = nc.dram_tensor("w1xg", [128, NCH, A], f32, kind="ExternalInput")
    w1m_d = nc.dram_tensor("w1mg", [128, NCH, A], f32, kind="ExternalInput")
    w1s_d = nc.dram_tensor("w1sg", [128, NCH, A], f32, kind="ExternalInput")
    w2_d = nc.dram_tensor("w2g", [A, NCH, 128], bf16, kind="ExternalInput")
    b1t_d = nc.dram_tensor("b1T", [1, A], f32, kind="ExternalInput")
    s1_d = nc.dram_tensor("s1v", [A, 1], f32, kind="ExternalInput")
    sh1_d = nc.dram_tensor("sh1v", [A, 1], f32, kind="ExternalInput")
    s2_d = nc.dram_tensor("s2c", [128, NCH], f32, kind="ExternalInput")
    s2b2_d = nc.dram_tensor("s2b2c", [128, NCH], f32, kind="ExternalInput")
    out_d = nc.dram_tensor("out_asp", [B_LOC, 2 * C], f32, kind="ExternalOutput")

    with tile.TileContext(nc) as tc:
        with (
            tc.tile_pool(name="consts", bufs=1) as consts,
            tc.tile_pool(name="xf", bufs=2) as xfp,
            tc.tile_pool(name="hp", bufs=2) as hp,
            tc.tile_pool(name="cp", bufs=3) as cp,
            tc.tile_pool(name="st", bufs=2) as st,
            tc.tile_pool(name="ph", bufs=1, space="PSUM") as php,
            tc.tile_pool(name="pa", bufs=2, space="PSUM") as pap,
            tc.tile_pool(name="pv", bufs=2, space="PSUM") as pvp,
        ):
            w1x_sb = consts.tile([128, NCH, A], f32)
            nc.sync.dma_start(out=w1x_sb, in_=w1x_d[:, :, :])
            w1m_sb = consts.tile([128, NCH, A], f32)
            nc.sync.dma_start(out=w1m_sb, in_=w1m_d[:, :, :])
            w1s_sb = consts.tile([128, NCH, A], f32)
            nc.sync.dma_start(out=w1s_sb, in_=w1s_d[:, :, :])
            w2_sb = consts.tile([A, NCH, 128], bf16)
            nc.sync.dma_start(out=w2_sb, in_=w2_d[:, :, :])
            b1t_sb = consts.tile([1, A], f32)
            nc.sync.dma_start(out=b1t_sb, in_=b1t_d[:, :])
            s1_sb = consts.tile([A, 1], f32)
            nc.sync.dma_start(out=s1_sb, in_=s1_d[:, :])
            sh1_sb = consts.tile([A, 1], f32)
            nc.sync.dma_start(out=sh1_sb, in_=sh1_d[:, :])
            s2_sb = consts.tile([128, NCH], f32)
            nc.sync.dma_start(out=s2_sb, in_=s2_d[:, :])
            s2b2_sb = consts.tile([128, NCH], f32)
            nc.sync.dma_start(out=s2b2_sb, in_=s2b2_d[:, :])
            onesT = consts.tile([1, T], f32)
            nc.vector.memset(onesT, 1.0)
            neg1 = consts.tile([128, 1], f32)
            nc.vector.memset(neg1, -1.0)

            nch4 = B_LOC * NCH
            sva = consts.tile([128, nch4], f32)
            m1a = consts.tile([128, nch4], f32)
            m2a = consts.tile([128, nch4], f32)

            def phase_a(b):
                """Load sample b and compute its x-stats (mean, var, sd)."""
                xf = xfp.tile([128, NCH, T], f32, name="xf", tag="xf")
                nc.sync.dma_start(
                    out=xf, in_=x_d[b].rearrange("(p j) t -> p j t", j=NCH)
                )
                mv = st.tile([128, NCH, 2], f32, name="mv", tag="mv")
                if SA > 0:
                    sxa = st.tile([128, SA], f32, name="sxa", tag="sxa")
                    sxxa = st.tile([128, SA], f32, name="sxxa", tag="sxxa")
                for j in range(NCH):
                    if j < SA:
                        tr1 = cp.tile([128, T], bf16, name="tr1", tag="xsq")
                        nc.scalar.activation(out=tr1, in_=xf[:, j, :],
                                             func=ACTF.Identity,
                                             accum_out=sxa[:, j:j + 1])
                        tr2 = cp.tile([128, T], bf16, name="tr2", tag="xsq")
                        nc.scalar.activation(out=tr2, in_=xf[:, j, :],
                                             func=ACTF.Square,
                                             accum_out=sxxa[:, j:j + 1])
                    else:
                        st6 = st.tile([128, 2, 6], f32, name="st6", tag="st6",
                                      bufs=3)
                        nc.vector.bn_stats(out=st6[:, 0, :], in_=xf[:, j, 0:TS0])
                        nc.vector.bn_stats(out=st6[:, 1, :], in_=xf[:, j, TS0:T])
                        nc.vector.bn_aggr(out=mv[:, j, :], in_=st6)
                if SA > 0:
                    # mean = sx/T ; var_pop = sxx/T - mean^2
                    nc.vector.tensor_scalar(out=mv[:, 0:SA, 0], in0=sxa,
                                            scalar1=1.0 / T, scalar2=None,
                                            op0=ALU.mult)
                    msq = st.tile([128, SA], f32, name="msq", tag="msq")
                    nc.vector.tensor_tensor(out=msq, in0=mv[:, 0:SA, 0],
                                            in1=mv[:, 0:SA, 0], op=ALU.mult)
                    nc.vector.scalar_tensor_tensor(
                        out=mv[:, 0:SA, 1], in0=sxxa, scalar=1.0 / T, in1=msq,
                        op0=ALU.mult, op1=ALU.subtract)

                # sd = sqrt(clip(var_pop*T/(T-1), CLAMP))
                sdsq = st.tile([128, NCH], f32, name="sdsq", tag="sdsq")
                nc.vector.tensor_scalar(out=sdsq, in0=mv[:, :, 1],
                                        scalar1=float(T) / (T - 1),
                                        scalar2=CLAMP,
                                        op0=ALU.mult, op1=ALU.max)
                rqs = _rsqrt_newton(nc, mybir, st, sdsq, 1, tag="sdn")
                sd = st.tile([128, NCH], f32, name="sd", tag="sd")
                nc.vector.tensor_tensor(out=sd, in0=sdsq, in1=rqs, op=ALU.mult)
                return xf, mv, sd

            # software pipeline: sample b+1's load+stats are emitted before
            # sample b's heavy phases so PE never waits on fresh stats
            staged = phase_a(0)
            for b in range(B_LOC):
                xf, mv, sd = staged
                if b + 1 < B_LOC:
                    staged = phase_a(b + 1)

                # ---- stats matvec: hvT[1, A] = mu@W1m + sd@W1s (M=1) ----
                hvt = pvp.tile([1, A], f32, name="hvt", tag="hvt")
                for j in range(NCH):
                    nc.tensor.matmul(hvt[0:1, :], mv[:, j, 0:1], w1m_sb[:, j, :],
                                     start=(j == 0), stop=False)
                for j in range(NCH):
                    nc.tensor.matmul(hvt[0:1, :], sd[:, j:j + 1], w1s_sb[:, j, :],
                                     start=False, stop=(j == NCH - 1))
                hvb = st.tile([1, A], f32, name="hvb", tag="hvb")
                nc.vector.tensor_tensor(out=hvb, in0=hvt[0:1, :],
                                        in1=b1t_sb[0:1, :], op=ALU.add)

                # ---- mm1 (+ K=1 bias matmul) and h ----
                ph = php.tile([128, 1024], f32, name="ph", tag="ph")
                nc.tensor.matmul(ph[:, 0:TS0], hvb[0:1, :], onesT[0:1, 0:TS0],
                                 start=True, stop=False)
                nc.tensor.matmul(ph[:, TS0:T], hvb[0:1, :], onesT[0:1, 0:T - TS0],
                                 start=True, stop=False)
                for j in range(NCH):
                    nc.tensor.matmul(ph[:, 0:TS0], w1x_sb[:, j, :],
                                     xf[:, j, 0:TS0],
                                     start=False, stop=(j == NCH - 1))
                    nc.tensor.matmul(ph[:, TS0:T], w1x_sb[:, j, :],
                                     xf[:, j, TS0:T],
                                     start=False, stop=(j == NCH - 1))
                r1 = hp.tile([128, T], bf16, name="r1", tag="r1")
                nc.scalar.activation(out=r1, in_=ph[:, 0:T], func=ACTF.Relu)
                h = hp.tile([128, T], bf16, name="h", tag="h")
                nc.scalar.activation(out=h, in_=r1, func=ACTF.Tanh,
                                     bias=sh1_sb[:, 0:1], scale=s1_sb[:, 0:1])

                # ---- phase C ----
                for j in range(NCH):
                    col = b * NCH + j
                    pa = pap.tile([128, 1024], f32, name="pa", tag="pa")
                    nc.tensor.matmul(pa[:, 0:TS0], w2_sb[:, j, :], h[:, 0:TS0],
                                     start=True, stop=True)
                    nc.tensor.matmul(pa[:, TS0:T], w2_sb[:, j, :], h[:, TS0:T],
                                     start=True, stop=True)
                    # e = exp(s2*a + s2*b2); p_raw = max(e, 1) folded below
                    e = cp.tile([128, T], bf16, name="e", tag="e")
                    nc.scalar.activation(out=e, in_=pa[:, 0:T], func=ACTF.Exp,
                                         bias=s2b2_sb[:, j:j + 1],
                                         scale=s2_sb[:, j:j + 1])
                    if j < NCH - SB:
                        # s on DVE: sva col = sum(max(e,1))
                        ptr = cp.tile([128, T], bf16, name="ptr", tag="xsq")
                        nc.vector.tensor_scalar(
                            out=ptr, in0=e, scalar1=1.0, scalar2=None,
                            op0=ALU.max, op1=ALU.add,
                            accum_out=sva[:, col:col + 1])
                    else:
                        # s on ACT: sva col = sum(relu(e-1)) = s - T
                        ptr = cp.tile([128, T], bf16, name="ptr", tag="xsq")
                        nc.scalar.activation(out=ptr, in_=e, func=ACTF.Relu,
                                             bias=neg1[:, 0:1],
                                             accum_out=sva[:, col:col + 1])
                    t1 = cp.tile([128, T], bf16, name="t1", tag="t1")
                    nc.vector.scalar_tensor_tensor(
                        out=t1, in0=e, scalar=1.0, in1=xf[:, j, :],
                        op0=ALU.max, op1=ALU.mult,
                        accum_out=m1a[:, col:col + 1])
                    t2 = cp.tile([128, T], bf16, name="t2", tag="t2")
                    nc.vector.scalar_tensor_tensor(
                        out=t2, in0=t1, scalar=1.0, in1=xf[:, j, :],
                        op0=ALU.mult, op1=ALU.mult,
                        accum_out=m2a[:, col:col + 1])

            # ---- finals (batched) ----
            if SB > 0:
                for b in range(B_LOC):
                    c0 = b * NCH + (NCH - SB)
                    c1 = (b + 1) * NCH
                    nc.vector.tensor_scalar(out=sva[:, c0:c1],
                                            in0=sva[:, c0:c1],
                                            scalar1=float(T), scalar2=None,
                                            op0=ALU.add)
            rs = consts.tile([128, nch4], f32)
            nc.vector.reciprocal(out=rs, in_=sva)
            mua = consts.tile([128, nch4], f32)
            nc.vector.tensor_tensor(out=mua, in0=m1a, in1=rs, op=ALU.mult)
            e2a = consts.tile([128, nch4], f32)
            nc.vector.tensor_tensor(out=e2a, in0=m2a, in1=rs, op=ALU.mult)
            msqa = consts.tile([128, nch4], f32)
            nc.vector.tensor_tensor(out=msqa, in0=mua, in1=mua, op=ALU.mult)
            dv = consts.tile([128, nch4], f32)
            nc.vector.tensor_tensor(out=dv, in0=e2a, in1=msqa, op=ALU.subtract)
            nc.vector.tensor_scalar(out=dv, in0=dv, scalar1=CLAMP, scalar2=None,
                                    op0=ALU.max)
            rqf = _rsqrt_newton(nc, mybir, st, dv, 2, tag="sgn")
            sga = consts.tile([128, nch4], f32)
            nc.vector.tensor_tensor(out=sga, in0=dv, in1=rqf, op=ALU.mult)

            for b in range(B_LOC):
                nc.sync.dma_start(
                    out=out_d[b, 0:C].rearrange("(p j) -> p j", j=NCH),
                    in_=mua[:, b * NCH:(b + 1) * NCH],
                )
                nc.sync.dma_start(
                    out=out_d[b, C:2 * C].rearrange("(p j) -> p j", j=NCH),
                    in_=sga[:, b * NCH:(b + 1) * NCH],
                )

    nc.compile()
    return nc


def _prep_params(W1, b1, gamma1, beta1, mean1, var1, W2, b2, gamma2, beta2,
                 mean2, var2):
    import ml_dtypes

    bf16 = ml_dtypes.bfloat16
    f32 = np.float32
    W1 = np.asarray(W1, f32)
    W2 = np.asarray(W2, f32)
    s1 = np.asarray(gamma1, f32) / np.sqrt(np.asarray(var1, f32) + BN_EPS)
    sh1 = np.asarray(beta1, f32) - np.asarray(mean1, f32) * s1
    s2 = np.asarray(gamma2, f32) / np.sqrt(np.asarray(var2, f32) + BN_EPS)
    assert (s2 > 0).all(), "kernel fast path requires positive bn2 scale"
    b2 = np.asarray(b2, f32)

    w1xg = np.ascontiguousarray(W1[:, :C].T.reshape(128, NCH, A))
    w1mg = np.ascontiguousarray(W1[:, C:2 * C].T.reshape(128, NCH, A))
    w1sg = np.ascontiguousarray(W1[:, 2 * C:].T.reshape(128, NCH, A))
    w2g = np.ascontiguousarray(
        W2.reshape(128, NCH, A).transpose(2, 1, 0)).astype(bf16)
    return {
        "w1xg": w1xg,
        "w1mg": w1mg,
        "w1sg": w1sg,
        "w2g": w2g,
        "b1T": np.asarray(b1, f32).reshape(1, A),
        "s1v": s1.reshape(A, 1),
        "sh1v": sh1.reshape(A, 1),
        "s2c": np.ascontiguousarray(s2.reshape(128, NCH)),
        "s2b2c": np.ascontiguousarray((s2 * b2).reshape(128, NCH)),
    }


def kernel(x, W1, b1, gamma1, beta1, mean1, var1,
           W2, b2, gamma2, beta2, mean2, var2):
    global LAST_EXEC_NS
    from concourse.bass_utils import run_bass_kernel_spmd

    if "nc" not in _BUILT:
        _BUILT["nc"] = build_kernel()
    nc = _BUILT["nc"]

    x = np.ascontiguousarray(np.asarray(x, np.float32))
    params = _prep_params(W1, b1, gamma1, beta1, mean1, var1,
                          W2, b2, gamma2, beta2, mean2, var2)
    in_maps = []
    for i in range(N_CORES):
        m = dict(params)
        m["x_in"] = np.ascontiguousarray(x[i * B_LOC:(i + 1) * B_LOC])
        in_maps.append(m)

    res = run_bass_kernel_spmd(nc, in_maps, list(range(N_CORES)), trace=TRACE)
    LAST_EXEC_NS = res.exec_time_ns
    out = np.concatenate(
        [res.results[i]["out_asp"] for i in range(N_CORES)], axis=0
    )
    return out.astype(np.float32)


# revision 6
# speedup vs baseline: 1.1977x; 1.0541x over previous
"""ASP (attentive statistics pooling) block kernel for Trainium2, 8 cores.

Shapes hardcoded for nn_ASPBlock: x [32, 1536, 800] f32, W1 [128, 4608],
W2 [1536, 128], A=128. Data-parallel over batch: 4 samples per core.

Channel layout is "dense": channel c lives at (partition p, chunk j) with
c = p*12 + j, so each partition's 12 channels are contiguous in DRAM and
one DMA descriptor moves 38.4KB.

The emission is a 3-deep software pipeline interleaved at chunk level so
the in-order engines never wait on each other:
  iteration b, chunk j: [mm2/exp/sum/moments for (b,j)] + [x-stats (b+2,j)]
  with sample b+1's stats-matvec and mm1 slotted between chunk groups.
"""

import numpy as np

B, C, T, A = 32, 1536, 800, 128
N_CORES = 8
B_LOC = B // N_CORES          # 4 samples per core
NCH = C // 128                # 12 chunks; channel c = p*NCH + j
TS0 = 512
BN_EPS = 1e-5
CLAMP = 1e-4
RSQRT_MAGIC = 0x5F3759DF

SA = 4      # chunks/sample with x-stats on ACT (rest: DVE bn_stats)
SB = 10     # chunks/sample with softmax-denominator on ACT (rest: DVE)

TRACE = False
LAST_EXEC_NS = None
_BUILT = {}


def build_kernel():
    import concourse.bacc as bacc
    import concourse.tile as tile
    from concourse import mybir

    f32 = mybir.dt.float32
    bf16 = mybir.dt.bfloat16
    i32 = mybir.dt.int32
    ALU = mybir.AluOpType
    ACTF = mybir.ActivationFunctionType

    nc = bacc.Bacc()

    x_d = nc.dram_tensor("x_in", [B_LOC, C, T], f32, kind="ExternalInput")
    w1x_d = nc.dram_tensor("w1xg", [128, NCH, A], f32, kind="ExternalInput")
    w1m_d = nc.dram_tensor("w1mg", [128, NCH, A], f32, kind="ExternalInput")
    w1s_d = nc.dram_tensor("w1sg", [128, NCH, A], f32, kind="ExternalInput")
    w2_d = nc.dram_tensor("w2g", [A, NCH, 128], bf16, kind="ExternalInput")
    b1t_d = nc.dram_tensor("b1T", [1, A], f32, kind="ExternalInput")
    s1_d = nc.dram_tensor("s1v", [A, 1], f32, kind="ExternalInput")
    sh1_d = nc.dram_tensor("sh1v", [A, 1], f32, kind="ExternalInput")
    s2_d = nc.dram_tensor("s2c", [128, NCH], f32, kind="ExternalInput")
    s2b2_d = nc.dram_tensor("s2b2c", [128, NCH], f32, kind="ExternalInput")
    out_d = nc.dram_tensor("out_asp", [B_LOC, 2 * C], f32, kind="ExternalOutput")

    with tile.TileContext(nc) as tc:
        with (
            tc.tile_pool(name="consts", bufs=1) as consts,
            tc.tile_pool(name="xf", bufs=3) as xfp,
            tc.tile_pool(name="hp", bufs=2) as hp,
            tc.tile_pool(name="cp", bufs=3) as cp,
            tc.tile_pool(name="st", bufs=3) as st,
            tc.tile_pool(name="ph", bufs=1, space="PSUM") as php,
            tc.tile_pool(name="pa", bufs=2, space="PSUM") as pap,
            tc.tile_pool(name="pv", bufs=1, space="PSUM") as pvp,
        ):
            w1x_sb = consts.tile([128, NCH, A], f32)
            nc.sync.dma_start(out=w1x_sb, in_=w1x_d[:, :, :])
            w1m_sb = consts.tile([128, NCH, A], f32)
            nc.sync.dma_start(out=w1m_sb, in_=w1m_d[:, :, :])
            w1s_sb = consts.tile([128, NCH, A], f32)
            nc.sync.dma_start(out=w1s_sb, in_=w1s_d[:, :, :])
            w2_sb = consts.tile([A, NCH, 128], bf16)
            nc.sync.dma_start(out=w2_sb, in_=w2_d[:, :, :])
            b1t_sb = consts.tile([1, A], f32)
            nc.sync.dma_start(out=b1t_sb, in_=b1t_d[:, :])
            s1_sb = consts.tile([A, 1], f32)
            nc.sync.dma_start(out=s1_sb, in_=s1_d[:, :])
            sh1_sb = consts.tile([A, 1], f32)
            nc.sync.dma_start(out=sh1_sb, in_=sh1_d[:, :])
            s2_sb = consts.tile([128, NCH], f32)
            nc.sync.dma_start(out=s2_sb, in_=s2_d[:, :])
            s2b2_sb = consts.tile([128, NCH], f32)
            nc.sync.dma_start(out=s2b2_sb, in_=s2b2_d[:, :])
            onesT = consts.tile([1, T], f32)
            nc.vector.memset(onesT, 1.0)
            neg1 = consts.tile([128, 1], f32)
            nc.vector.memset(neg1, -1.0)
            magic = consts.tile([128, NCH], i32)
            nc.vector.memset(magic, RSQRT_MAGIC)
            magicw = consts.tile([128, B_LOC * NCH], i32)
            nc.vector.memset(magicw, RSQRT_MAGIC)

            nch4 = B_LOC * NCH
            sva = consts.tile([128, nch4], f32)
            m1a = consts.tile([128, nch4], f32)
            m2a = consts.tile([128, nch4], f32)

            state = [dict() for _ in range(B_LOC)]

            def rsqrt_newton(v, n_iters, tag, mg):
                y = st.tile(list(v.shape), f32, name=f"{tag}_y", tag=f"{tag}_y")
                nc.vector.tensor_scalar(out=y.bitcast(i32), in0=v.bitcast(i32),
                                        scalar1=1, scalar2=None,
                                        op0=ALU.arith_shift_right)
                nc.vector.tensor_tensor(out=y.bitcast(i32), in0=mg,
                                        in1=y.bitcast(i32), op=ALU.subtract)
                for it in range(n_iters):
                    t = st.tile(list(v.shape), f32, name=f"{tag}_t",
                                tag=f"{tag}_t")
                    nc.vector.tensor_tensor(out=t, in0=v, in1=y, op=ALU.mult)
                    nc.vector.tensor_tensor(out=t, in0=t, in1=y, op=ALU.mult)
                    nc.vector.tensor_scalar(out=t, in0=t, scalar1=-0.5,
                                            scalar2=1.5, op0=ALU.mult,
                                            op1=ALU.add)
                    nc.vector.tensor_tensor(out=y, in0=y, in1=t, op=ALU.mult)
                return y

            def s_load(b):
                xf = xfp.tile([128, NCH, T], f32, name="xf", tag="xf")
                nc.sync.dma_start(
                    out=xf, in_=x_d[b].rearrange("(p j) t -> p j t", j=NCH))
                mv = st.tile([128, NCH, 2], f32, name="mv", tag="mv")
                sxa = st.tile([128, max(SA, 1)], f32, name="sxa", tag="sxa")
                sxxa = st.tile([128, max(SA, 1)], f32, name="sxxa", tag="sxxa")
                state[b] = {"xf": xf, "mv": mv, "sxa": sxa, "sxxa": sxxa}

            def s_stat(b, j):
                xf, mv = state[b]["xf"], state[b]["mv"]
                if j < SA:
                    tr1 = cp.tile([128, T], bf16, name="tr1", tag="xsq")
                    nc.scalar.activation(out=tr1, in_=xf[:, j, :],
                                         func=ACTF.Identity,
                                         accum_out=state[b]["sxa"][:, j:j + 1])
                    tr2 = cp.tile([128, T], bf16, name="tr2", tag="xsq")
                    nc.scalar.activation(out=tr2, in_=xf[:, j, :],
                                         func=ACTF.Square,
                                         accum_out=state[b]["sxxa"][:, j:j + 1])
                else:
                    st6 = st.tile([128, 2, 6], f32, name="st6", tag="st6")
                    nc.vector.bn_stats(out=st6[:, 0, :], in_=xf[:, j, 0:TS0])
                    nc.vector.bn_stats(out=st6[:, 1, :], in_=xf[:, j, TS0:T])
                    nc.vector.bn_aggr(out=mv[:, j, :], in_=st6)

            def s_statfix(b):
                mv = state[b]["mv"]
                if SA > 0:
                    sxa, sxxa = state[b]["sxa"], state[b]["sxxa"]
                    nc.vector.tensor_scalar(out=mv[:, 0:SA, 0], in0=sxa,
                                            scalar1=1.0 / T, scalar2=None,
                                            op0=ALU.mult)
                    msq = st.tile([128, SA], f32, name="msq", tag="msq")
                    nc.vector.tensor_tensor(out=msq, in0=mv[:, 0:SA, 0],
                                            in1=mv[:, 0:SA, 0], op=ALU.mult)
                    nc.vector.scalar_tensor_tensor(
                        out=mv[:, 0:SA, 1], in0=sxxa, scalar=1.0 / T, in1=msq,
                        op0=ALU.mult, op1=ALU.subtract)
                sdsq = st.tile([128, NCH], f32, name="sdsq", tag="sdsq")
                nc.vector.tensor_scalar(out=sdsq, in0=mv[:, :, 1],
                                        scalar1=float(T) / (T - 1),
                                        scalar2=CLAMP, op0=ALU.mult,
                                        op1=ALU.max)
                y = rsqrt_newton(sdsq, 1, "sdn", magic)
                sd = st.tile([128, NCH], f32, name="sd", tag="sd")
                nc.vector.tensor_tensor(out=sd, in0=sdsq, in1=y, op=ALU.mult)
                state[b]["sd"] = sd

            def s_matvec(b):
                mv, sd = state[b]["mv"], state[b]["sd"]
                hvt = pvp.tile([1, A], f32, name="hvt", tag="hvt")
                for j in range(NCH):
                    nc.tensor.matmul(hvt[0:1, :], mv[:, j, 0:1],
                                     w1m_sb[:, j, :],
                                     start=(j == 0), stop=False)
                for j in range(NCH):
                    nc.tensor.matmul(hvt[0:1, :], sd[:, j:j + 1],
                                     w1s_sb[:, j, :],
                                     start=False, stop=(j == NCH - 1))
                hvb = st.tile([1, A], f32, name="hvb", tag="hvb")
                nc.vector.tensor_tensor(out=hvb, in0=hvt[0:1, :],
                                        in1=b1t_sb[0:1, :], op=ALU.add)
                state[b]["hvb"] = hvb

            def s_mm1(b, jlist):
                xf = state[b]["xf"]
                if "ph" not in state[b]:
                    ph = php.tile([128, 1024], f32, name="ph", tag="ph")
                    state[b]["ph"] = ph
                    hvb = state[b]["hvb"]
                    nc.tensor.matmul(ph[:, 0:TS0], hvb[0:1, :],
                                     onesT[0:1, 0:TS0], start=True, stop=False)
                    nc.tensor.matmul(ph[:, TS0:T], hvb[0:1, :],
                                     onesT[0:1, 0:T - TS0],
                                     start=True, stop=False)
                ph = state[b]["ph"]
                for j in jlist:
                    last = (j == NCH - 1)
                    nc.tensor.matmul(ph[:, 0:TS0], w1x_sb[:, j, :],
                                     xf[:, j, 0:TS0], start=False, stop=last)
                    nc.tensor.matmul(ph[:, TS0:T], w1x_sb[:, j, :],
                                     xf[:, j, TS0:T], start=False, stop=last)

            def s_h(b):
                ph = state[b].pop("ph")
                r1 = hp.tile([128, T], bf16, name="r1", tag="r1")
                nc.scalar.activation(out=r1, in_=ph[:, 0:T], func=ACTF.Relu)
                h = hp.tile([128, T], bf16, name="h", tag="h")
                nc.scalar.activation(out=h, in_=r1, func=ACTF.Tanh,
                                     bias=sh1_sb[:, 0:1], scale=s1_sb[:, 0:1])
                state[b]["h"] = h

            def s_c(b, j):
                xf, h = state[b]["xf"], state[b]["h"]
                col = b * NCH + j
                pa = pap.tile([128, 1024], f32, name="pa", tag="pa")
                nc.tensor.matmul(pa[:, 0:TS0], w2_sb[:, j, :], h[:, 0:TS0],
                                 start=True, stop=True)
                nc.tensor.matmul(pa[:, TS0:T], w2_sb[:, j, :], h[:, TS0:T],
                                 start=True, stop=True)
                e = cp.tile([128, T], bf16, name="e", tag="e")
                nc.scalar.activation(out=e, in_=pa[:, 0:T], func=ACTF.Exp,
                                     bias=s2b2_sb[:, j:j + 1],
                                     scale=s2_sb[:, j:j + 1])
                ptr = cp.tile([128, T], bf16, name="ptr", tag="xsq")
                if j < NCH - SB:
                    nc.vector.tensor_scalar(
                        out=ptr, in0=e, scalar1=1.0, scalar2=None,
                        op0=ALU.max, op1=ALU.add,
                        accum_out=sva[:, col:col + 1])
                else:
                    nc.scalar.activation(out=ptr, in_=e, func=ACTF.Relu,
                                         bias=neg1[:, 0:1],
                                         accum_out=sva[:, col:col + 1])
                t1 = cp.tile([128, T], bf16, name="t1", tag="t1")
                nc.vector.scalar_tensor_tensor(
                    out=t1, in0=e, scalar=1.0, in1=xf[:, j, :],
                    op0=ALU.max, op1=ALU.mult, accum_out=m1a[:, col:col + 1])
                t2 = cp.tile([128, T], bf16, name="t2", tag="t2")
                nc.vector.scalar_tensor_tensor(
                    out=t2, in0=t1, scalar=1.0, in1=xf[:, j, :],
                    op0=ALU.mult, op1=ALU.mult, accum_out=m2a[:, col:col + 1])

            # ---------------- pipeline schedule ----------------
            s_load(0)
            for j in range(NCH):
                s_stat(0, j)
            s_statfix(0)
            s_load(1)
            for j in range(NCH):
                s_stat(1, j)
            s_statfix(1)
            s_matvec(0)
            s_mm1(0, range(NCH))
            s_h(0)

            for b in range(B_LOC):
                nxt = b + 1 < B_LOC
                pre = b + 2 < B_LOC
                for j in range(NCH):
                    s_c(b, j)
                    if pre:
                        if j == 0:
                            s_load(b + 2)
                        s_stat(b + 2, j)
                    if nxt:
                        if j == 3:
                            s_matvec(b + 1)
                        elif j == 5:
                            s_mm1(b + 1, range(0, 6))
                        elif j == 8:
                            s_mm1(b + 1, range(6, NCH))
                        elif j == 9:
                            s_h(b + 1)
                if pre:
                    s_statfix(b + 2)

            # ---------------- finals (batched) ----------------
            if SB > 0:
                for b in range(B_LOC):
                    c0 = b * NCH + (NCH - SB)
                    c1 = (b + 1) * NCH
                    nc.vector.tensor_scalar(out=sva[:, c0:c1],
                                            in0=sva[:, c0:c1],
                                            scalar1=float(T), scalar2=None,
                                            op0=ALU.add)
            rs = consts.tile([128, nch4], f32)
            nc.vector.reciprocal(out=rs, in_=sva)
            mua = consts.tile([128, nch4], f32)
            nc.vector.tensor_tensor(out=mua, in0=m1a, in1=rs, op=ALU.mult)
            e2a = consts.tile([128, nch4], f32)
            nc.vector.tensor_tensor(out=e2a, in0=m2a, in1=rs, op=ALU.mult)
            msqa = consts.tile([128, nch4], f32)
            nc.vector.tensor_tensor(out=msqa, in0=mua, in1=mua, op=ALU.mult)
            dv = consts.tile([128, nch4], f32)
            nc.vector.tensor_tensor(out=dv, in0=e2a, in1=msqa, op=ALU.subtract)
            nc.vector.tensor_scalar(out=dv, in0=dv, scalar1=CLAMP, scalar2=None,
                                    op0=ALU.max)

            yf = consts.tile([128, nch4], f32)
            nc.vector.tensor_scalar(out=yf.bitcast(i32), in0=dv.bitcast(i32),
                                    scalar1=1, scalar2=None,
                                    op0=ALU.arith_shift_right)
            nc.vector.tensor_tensor(out=yf.bitcast(i32), in0=magicw,
                                    in1=yf.bitcast(i32), op=ALU.subtract)
            for _ in range(2):
                tn = consts.tile([128, nch4], f32, name="tn", tag="tn", bufs=2)
                nc.vector.tensor_tensor(out=tn, in0=dv, in1=yf, op=ALU.mult)
                nc.vector.tensor_tensor(out=tn, in0=tn, in1=yf, op=ALU.mult)
                nc.vector.tensor_scalar(out=tn, in0=tn, scalar1=-0.5,
                                        scalar2=1.5, op0=ALU.mult, op1=ALU.add)
                nc.vector.tensor_tensor(out=yf, in0=yf, in1=tn, op=ALU.mult)
            sga = consts.tile([128, nch4], f32)
            nc.vector.tensor_tensor(out=sga, in0=dv, in1=yf, op=ALU.mult)

            for b in range(B_LOC):
                nc.sync.dma_start(
                    out=out_d[b, 0:C].rearrange("(p j) -> p j", j=NCH),
                    in_=mua[:, b * NCH:(b + 1) * NCH],
                )
                nc.sync.dma_start(
                    out=out_d[b, C:2 * C].rearrange("(p j) -> p j", j=NCH),
                    in_=sga[:, b * NCH:(b + 1) * NCH],
                )

    nc.compile()
    return nc


def _prep_params(W1, b1, gamma1, beta1, mean1, var1, W2, b2, gamma2, beta2,
                 mean2, var2):
    import ml_dtypes

    bf16 = ml_dtypes.bfloat16
    f32 = np.float32
    W1 = np.asarray(W1, f32)
    W2 = np.asarray(W2, f32)
    s1 = np.asarray(gamma1, f32) / np.sqrt(np.asarray(var1, f32) + BN_EPS)
    sh1 = np.asarray(beta1, f32) - np.asarray(mean1, f32) * s1
    s2 = np.asarray(gamma2, f32) / np.sqrt(np.asarray(var2, f32) + BN_EPS)
    assert (s2 > 0).all(), "kernel fast path requires positive bn2 scale"
    b2 = np.asarray(b2, f32)

    w1xg = np.ascontiguousarray(W1[:, :C].T.reshape(128, NCH, A))
    w1mg = np.ascontiguousarray(W1[:, C:2 * C].T.reshape(128, NCH, A))
    w1sg = np.ascontiguousarray(W1[:, 2 * C:].T.reshape(128, NCH, A))
    w2g = np.ascontiguousarray(
        W2.reshape(128, NCH, A).transpose(2, 1, 0)).astype(bf16)
    return {
        "w1xg": w1xg,
        "w1mg": w1mg,
        "w1sg": w1sg,
        "w2g": w2g,
        "b1T": np.asarray(b1, f32).reshape(1, A),
        "s1v": s1.reshape(A, 1),
        "sh1v": sh1.reshape(A, 1),
        "s2c": np.ascontiguousarray(s2.reshape(128, NCH)),
        "s2b2c": np.ascontiguousarray((s2 * b2).reshape(128, NCH)),
    }


def kernel(x, W1, b1, gamma1, beta1, mean1, var1,
           W2, b2, gamma2, beta2, mean2, var2):
    global LAST_EXEC_NS
    from concourse.bass_utils import run_bass_kernel_spmd

    if "nc" not in _BUILT:
        _BUILT["nc"] = build_kernel()
    nc = _BUILT["nc"]

    x = np.ascontiguousarray(np.asarray(x, np.float32))
    params = _prep_params(W1, b1, gamma1, beta1, mean1, var1,
                          W2, b2, gamma2, beta2, mean2, var2)
    in_maps = []
    for i in range(N_CORES):
        m = dict(params)
        m["x_in"] = np.ascontiguousarray(x[i * B_LOC:(i + 1) * B_LOC])
        in_maps.append(m)

    res = run_bass_kernel_spmd(nc, in_maps, list(range(N_CORES)), trace=TRACE)
    LAST_EXEC_NS = res.exec_time_ns
    out = np.concatenate(
        [res.results[i]["out_asp"] for i in range(N_CORES)], axis=0
    )
    return out.astype(np.float32)
